# revision 43
# baseline (speedup 1.0000x reference)
# AdaAttN (no-conv) Trainium2 kernel, SPMD over 8 NeuronCores.
#
# Problem (hardcoded shapes): inputs c_x, s_x, c_1x, s_1x all (4, 512, 64, 64) f32.
#   Q = IN(c_1x) as (b, hw, c);  K = IN(s_1x) as (b, c, hw);  V = s_x as (b, hw, c)
#   A = softmax(Q@K, axis=-1)        (NO 1/sqrt(d) scale -> logits ~ N(0, 512))
#   M = A@V ; Var = A@(V*V) - M^2 ; S = sqrt(clip(Var, 1e-6))
#   out = S * IN(c_x) + M  as (b, c, h, w)
#
# Sharding: 2 cores per sample (b=4 -> 8 cores). Each core handles 2048 of the
# 4096 query tokens but needs full K/V (all 4096 keys). Host "rolls" the token
# axis of c_1x / c_x for odd cores so every core's queries are columns [0:2048]
# of its own input copy (instance-norm stats are permutation invariant). s_x is
# passed host-transposed ([hw, c]) so the PV weights need no on-device
# transpose. All inputs are shipped fp16 (compute is fp16 anyway; halves the
# startup DMA; validated rel err 0.0051 vs 0.0047 with f32 inputs, gate 2e-2).
# No cross-core collectives. Per-core output is [2048 tokens, 512 ch] f32;
# host transposes/reassembles.
#
# Per Q-tile (128 queries):
#   scores[128,4096] f32 = QT^T@K  (PSUM, 8x512 chunks; ScalarE copies to SBUF,
#   DVE takes per-chunk maxes from PSUM in parallel with the QK matmuls)
#   P fp16 = exp(scores - max) in one ACT op (bias=-max/partition), Z via
#   accum_out; P^T via PE transpose (fp16, 4 blocks packed per PSUM bank,
#   DVE copies out); EV = P^T.T @ V, EV2 = P^T.T @ V^2 (32 k-blocks into PSUM)
#   M = EV/Z ; Var = EV2/Z - M^2 ; S = exp(0.5*ln(clip(Var)))  [single ACT
#   table set: natural_log_exp_and_others -> no table thrash]
#   out[q, c] = S * ncxT + M  -> DMA
import numpy as np

_CACHE: dict = {}

C = 512
HW = 4096
QH = 2048  # queries per core
CB = 4  # channel blocks of 128
KC = 8  # key chunks of 512
KB = 32  # key blocks of 128
NQT = 16  # query tiles of 128 per core
EPS_IN = 1e-5
EPS_VAR = 1e-6


def _patched_insert_act_table_loads(self):
    """All activation funcs used here live in natural_log_exp_and_others, so a
    single table load up front replaces the per-canonical-set thrash (56 loads
    / ~75us of InstLoadActFuncSet) that the stock inserter produces."""
    import concourse.mybir as mybir
    from concourse.hw_specs import get_activation_tables

    tables = get_activation_tables(self.m.arch)
    names = list(tables.keys())
    set_name = "natural_log_exp_and_others"
    set_id = names.index(set_name)
    allowed = tables[set_name]
    used = set()
    for b in self.main_func.blocks:
        for i in b.instructions:
            if isinstance(i, mybir.InstActivation):
                used.add(i.func)
    if not used:
        return
    assert used <= allowed, f"activation funcs {used - allowed} not in {set_name}"
    for blk in self.main_func.blocks:
        for idx, inst in enumerate(blk.instructions):
            if isinstance(inst, mybir.InstActivation):
                load = mybir.InstLoadActFuncSet(
                    name=self.get_next_instruction_name(),
                    ins=[],
                    outs=[],
                    act_func_set_id=set_id,
                )
                load.engine = mybir.EngineType.Activation
                self.register_instruction(load)
                blk.instructions.insert(idx, load)
                return


def _build():
    import types

    from concourse import bacc
    import concourse.mybir as mybir
    import concourse.tile as tile
    from concourse.masks import make_identity

    f32 = mybir.dt.float32
    f16 = mybir.dt.float16
    AF = mybir.ActivationFunctionType
    OP = mybir.AluOpType
    AX = mybir.AxisListType

    nc = bacc.Bacc(None, target_bir_lowering=False)
    nc.insert_act_table_loads = types.MethodType(_patched_insert_act_table_loads, nc)
    d_c1x = nc.dram_tensor("c1x", [C, HW], f16, kind="ExternalInput")
    d_s1x = nc.dram_tensor("s1x", [C, HW], f16, kind="ExternalInput")
    d_sxt = nc.dram_tensor("sxt", [HW, C], f16, kind="ExternalInput")
    d_cx = nc.dram_tensor("cx", [C, HW], f16, kind="ExternalInput")
    d_out = nc.dram_tensor("out", [QH, C], f32, kind="ExternalOutput")

    with tile.TileContext(nc) as tc:
        with (
            tc.tile_pool(name="const", bufs=1) as constp,
            tc.tile_pool(name="persist", bufs=1) as persist,
            tc.tile_pool(name="big", bufs=1) as bigp,
            tc.tile_pool(name="h16a", bufs=2) as h16a,
            tc.tile_pool(name="h16b", bufs=2) as h16b,
            tc.tile_pool(name="epi", bufs=1) as epi,
            tc.tile_pool(name="small", bufs=4) as small,
            tc.tile_pool(name="psum_s", bufs=3, space="PSUM") as psum_s,
            tc.tile_pool(name="psum_t", bufs=3, space="PSUM") as psum_t,
            tc.tile_pool(name="psum_mv", bufs=1, space="PSUM") as psum_mv,
        ):
            ident = constp.tile([128, 128], f16)
            make_identity(nc, ident[:])
            eps_in = constp.tile([128, 1], f32)
            nc.gpsimd.memset(eps_in[:], EPS_IN)

            # persistent fp16 operands (split per channel-block so Tile's
            # per-tile dependency tracking doesn't serialize the prep DMAs)
            K_t = [persist.tile([128, HW], f16, tag=f"K{cb}", name=f"K{cb}") for cb in range(CB)]
            QT_t = [persist.tile([128, QH], f16, tag=f"QT{cb}", name=f"QT{cb}") for cb in range(CB)]
            W_t = [persist.tile([128, 8, 1024], f16, tag=f"W{g}", name=f"W{g}") for g in range(4)]
            ncxT = persist.tile([128, NQT, C], f16)  # normalized c_x   [q, c]
            ncxh_all = persist.tile([128, CB, QH], f16)  # cx prep staging

            def finish_norm(mean, var, raw, dst, ncols):
                """rstd = exp(-0.5*ln(var+eps)) (ln/exp table set); then
                dst = (raw - mean) * rstd via ACT Identity."""
                lnv = small.tile([128, 1], f32, tag="lnv")
                nc.scalar.activation(lnv[:], var, AF.Ln, bias=eps_in[:])
                rstd = small.tile([128, 1], f32, tag="rstd")
                nc.scalar.activation(rstd[:], lnv[:], AF.Exp, scale=-0.5)
                negb = small.tile([128, 1], f32, tag="negb")
                nc.vector.tensor_scalar(
                    negb[:], mean, rstd[:], -1.0, op0=OP.mult, op1=OP.mult
                )
                nc.scalar.activation(
                    dst, raw[:, 0:ncols], AF.Identity, bias=negb[:], scale=rstd[:]
                )

            def norm_prep(dram, cb, raw, dst, ncols):
                """bn_stats variant (keeps DVE cost off the A-tag pool; used
                for cx where the critical path doesn't matter)."""
                nc.sync.dma_start(raw, dram[cb * 128 : (cb + 1) * 128, :])
                stv = small.tile([128, 8, 6], f32, tag="stats")
                st3 = raw.rearrange("p (n f) -> p n f", f=512)
                for i in range(8):
                    nc.vector.bn_stats(stv[:, i, :], st3[:, i, :])
                mv = small.tile([128, 2], f32, tag="mv")
                nc.vector.bn_aggr(mv[:], stv[:])
                finish_norm(mv[:, 0:1], mv[:, 1:2], raw, dst, ncols)

            def rstd_of(mv, tag):
                lnv = small.tile([128, 1], f32, tag="lnv")
                nc.scalar.activation(lnv[:], mv[:, 1:2], AF.Ln, bias=eps_in[:])
                r = small.tile([128, 1], f32, tag=tag)
                nc.scalar.activation(r[:], lnv[:], AF.Exp, scale=-0.5)
                return r

            # ---- prep: K stays RAW; its normalization folds into Q ----------
            # softmax(Q_hat^T K_hat) where K_hat = (K - mu_k)*rstd_k: the mu_k
            # term contributes a per-row constant to the logits (cancels in
            # softmax), and rstd_k is per-channel on the contraction axis, so
            # it folds into Q's normalize scale: Q2 = (c1x - mu_q) * rstd_q *
            # rstd_k. K is used straight from DMA; only its variance is needed.
            for cb in range(CB):
                nc.sync.dma_start(
                    K_t[cb][:], d_s1x[cb * 128 : (cb + 1) * 128, :]
                )
                wflat = W_t[cb][:].rearrange("p a b -> p (a b)")
                c1raw = wflat[:, 0:HW]
                nc.sync.dma_start(c1raw, d_c1x[cb * 128 : (cb + 1) * 128, :])
                # c1x moments split across engines into disjoint scratch
                # (ncxh_all is idle until cx prep): Sum(x) on ACT Identity,
                # Sum(x^2) on DVE scalar_tensor_tensor at 2x. Keeping c1raw
                # read-only lets the QT normalize proceed without false deps.
                ncxf = ncxh_all[:].rearrange("p a b -> p (a b)")
                trashA = ncxf[:, 0:HW]
                trashB_t = bigp.tile([128, HW], f16, tag="big", name=f"trashB{cb}")
                trashB = trashB_t[:]
                sums_q = small.tile([128, 1], f32, tag="sumsq1")
                nc.vector.tensor_scalar(
                    trashB, c1raw, 1.0, 0.0, op0=OP.mult, op1=OP.add,
                    accum_out=sums_q[:],
                )
                ssq_q = small.tile([128, 1], f32, tag="sumsq2")
                nc.scalar.activation(
                    trashA, c1raw, AF.Square, accum_out=ssq_q[:]
                )
                # K stats on DVE (only the variance is ever used)
                stv = small.tile([128, 8, 6], f32, tag="stats")
                k3 = K_t[cb][:].rearrange("p (n f) -> p n f", f=512)
                for i in range(8):
                    nc.vector.bn_stats(stv[:, i, :], k3[:, i, :])
                mv_k = small.tile([128, 2], f32, tag="mvk")
                nc.vector.bn_aggr(mv_k[:], stv[:])
                mean_q = small.tile([128, 1], f32, tag="meanq")
                nc.vector.tensor_scalar(
                    mean_q[:], sums_q[:], 1.0 / HW, 0.0, op0=OP.mult, op1=OP.add
                )
                msq_q = small.tile([128, 1], f32, tag="msqq")
                nc.vector.tensor_tensor(msq_q[:], mean_q[:], mean_q[:], op=OP.mult)
                var_q = small.tile([128, 1], f32, tag="varq")
                nc.vector.scalar_tensor_tensor(
                    var_q[:], ssq_q[:], 1.0 / HW, msq_q[:],
                    op0=OP.mult, op1=OP.subtract,
                )
                lnq = small.tile([128, 1], f32, tag="lnv")
                nc.scalar.activation(lnq[:], var_q[:], AF.Ln, bias=eps_in[:])
                rq = small.tile([128, 1], f32, tag="rstdq")
                nc.scalar.activation(rq[:], lnq[:], AF.Exp, scale=-0.5)
                rk = rstd_of(mv_k, "rstdk")
                rc = small.tile([128, 1], f32, tag="rc")
                nc.vector.tensor_tensor(rc[:], rq[:], rk[:], op=OP.mult)
                negb = small.tile([128, 1], f32, tag="negb")
                nc.vector.tensor_scalar(
                    negb[:], mean_q[:], rc[:], -1.0, op0=OP.mult, op1=OP.mult
                )
                nc.scalar.activation(
                    QT_t[cb][:], c1raw[:, 0:QH], AF.Identity,
                    bias=negb[:], scale=rc[:],
                )

            def emit_phase_a(t):
                scores = bigp.tile([128, HW], f32, tag="big")
                mpart = small.tile([128, KC], f32, tag="mpart")
                for kc in range(KC):
                    ps_s = psum_s.tile([128, 512], f32, tag="ps_s")
                    for cb in range(CB):
                        nc.tensor.matmul(
                            ps_s[:],
                            QT_t[cb][:, t * 128 : (t + 1) * 128],
                            K_t[cb][:, kc * 512 : (kc + 1) * 512],
                            start=(cb == 0),
                            stop=(cb == CB - 1),
                        )
                    nc.scalar.copy(scores[:, kc * 512 : (kc + 1) * 512], ps_s[:])
                    # per-chunk max straight from PSUM, overlapped with QK
                    nc.vector.reduce_max(mpart[:, kc : kc + 1], ps_s[:], axis=AX.X)
                negm = small.tile([128, 1], f32, tag="negm")
                nc.vector.reduce_max(negm[:], mpart[:], axis=AX.X, negate=True)
                P = h16a.tile([128, HW], f16, tag="A")
                zp = small.tile([128, 2], f32, tag="zp")
                for h in range(2):
                    nc.scalar.activation(
                        P[:, h * 2048 : (h + 1) * 2048],
                        scores[:, h * 2048 : (h + 1) * 2048],
                        AF.Exp, bias=negm[:], accum_out=zp[:, h : h + 1],
                    )
                z = small.tile([128, 1], f32, tag="z")
                nc.vector.reduce_sum(z[:], zp[:], axis=AX.X)
                rz = small.tile([128, 1], f32, tag="rz")
                nc.vector.reciprocal(rz[:], z[:])
                return P, rz

            def emit_phase_b1(P0, rz0, t0):
                """P^T transposes + PV matmuls; returns psum tiles."""
                PT = h16b.tile([128, KB, 128], f16, tag="B")
                for g in range(8):
                    pst = psum_t.tile([128, 512], f16, tag="ps_t")
                    p3 = pst[:].rearrange("p (j q) -> p j q", j=4)
                    for j in range(4):
                        kb = g * 4 + j
                        nc.tensor.transpose(
                            pst[:, j * 128 : (j + 1) * 128],
                            P0[:, kb * 128 : (kb + 1) * 128],
                            ident[:],
                        )
                    if g % 2 == 0:
                        nc.vector.tensor_copy(PT[:, g * 4 : (g + 1) * 4, :], p3)
                    else:
                        nc.scalar.copy(PT[:, g * 4 : (g + 1) * 4, :], p3)
                ps_m = psum_mv.tile([128, 512], f32, tag="ps_m")
                ps_v = psum_mv.tile([128, 512], f32, tag="ps_v")
                for kb in range(KB):
                    wt = W_t[kb // 8]
                    nc.tensor.matmul(
                        ps_m[:], PT[:, kb, :], wt[:, kb % 8, 0:512],
                        start=(kb == 0), stop=(kb == KB - 1),
                    )
                    nc.tensor.matmul(
                        ps_v[:], PT[:, kb, :], wt[:, kb % 8, 512:1024],
                        start=(kb == 0), stop=(kb == KB - 1),
                    )
                return ps_m, ps_v

            def emit_phase_b2(ps_m, ps_v, rz0, t0):
                """Epilogue: M = EV/Z ; Var = EV2/Z - M^2 ; S = sqrt(clip);
                out = S*ncxT + M -> DMA."""
                Mf = epi.tile([128, 512], f32, tag="Mf")
                nc.vector.tensor_scalar_mul(Mf[:], ps_m[:], rz0[:])
                T1 = epi.tile([128, 512], f32, tag="T1")
                nc.vector.tensor_scalar_mul(T1[:], ps_v[:], rz0[:])
                Msq = epi.tile([128, 512], f32, tag="Msq")
                nc.scalar.activation(Msq[:], Mf[:], AF.Square)
                nc.vector.tensor_tensor(T1[:], T1[:], Msq[:], op=OP.subtract)
                nc.vector.tensor_scalar_max(T1[:], T1[:], EPS_VAR)
                nc.scalar.activation(T1[:], T1[:], AF.Ln)
                Sv = epi.tile([128, 512], f32, tag="Sv")
                nc.scalar.activation(Sv[:], T1[:], AF.Exp, scale=0.5)
                nc.vector.tensor_tensor(Sv[:], Sv[:], ncxT[:, t0, :], op=OP.mult)
                outt = epi.tile([128, 512], f32, tag="Msq")
                nc.vector.tensor_tensor(outt[:], Sv[:], Mf[:], op=OP.add)
                nc.sync.dma_start(d_out[t0 * 128 : (t0 + 1) * 128, :], outt[:])

            def emit_prep_sxt():
                # sxt is [keys, ch] (host-transposed); no PE transposes needed.
                sxt4 = d_sxt[:].rearrange("(g b p) c -> g p b c", p=128, b=8)
                for g in range(4):
                    wst = h16b.tile([128, 8, 512], f16, tag="B")
                    nc.sync.dma_start(wst[:], sxt4[g])
                    vslice = W_t[g][:, :, 0:512]
                    nc.vector.tensor_copy(vslice, wst[:])
                    nc.vector.tensor_tensor(
                        W_t[g][:, :, 512:1024], vslice, vslice, op=OP.mult,
                    )

            def emit_prep_cx_norms():
                for cb in range(CB):
                    stage = h16a.tile([128, HW], f16, tag="A")
                    norm_prep(d_cx, cb, stage[:], ncxh_all[:, cb, :], QH)

            def emit_prep_cx_transposes():
                for cb in range(CB):
                    ncxh = ncxh_all[:, cb, :]
                    for g in range(4):
                        pst = psum_t.tile([128, 512], f16, tag="ps_t")
                        p3 = pst[:].rearrange("p (j q) -> p j q", j=4)
                        for j in range(4):
                            qt = g * 4 + j
                            nc.tensor.transpose(
                                pst[:, j * 128 : (j + 1) * 128],
                                ncxh[:, qt * 128 : (qt + 1) * 128],
                                ident[:],
                            )
                        if g % 2 == 0:
                            nc.vector.tensor_copy(
                                ncxT[:, g * 4 : (g + 1) * 4, cb * 128 : (cb + 1) * 128],
                                p3,
                            )
                        else:
                            nc.scalar.copy(
                                ncxT[:, g * 4 : (g + 1) * 4, cb * 128 : (cb + 1) * 128],
                                p3,
                            )

            # ---- pipeline: keep PE fed while sxt/cx prep DMAs stream in -----
            a0 = emit_phase_a(0)
            emit_prep_cx_norms()
            emit_prep_sxt()
            a1 = emit_phase_a(1)
            mv0 = emit_phase_b1(*a0, 0)
            emit_prep_cx_transposes()
            emit_phase_b2(*mv0, a0[1], 0)
            prev = a1
            for t in range(2, NQT + 1):
                cur = emit_phase_a(t) if t < NQT else None
                mv = emit_phase_b1(*prev, t - 1)
                emit_phase_b2(*mv, prev[1], t - 1)
                prev = cur

    nc.compile()
    return nc


def _get_nc():
    if "nc" not in _CACHE:
        _CACHE["nc"] = _build()
    return _CACHE["nc"]


def _prepare_in_maps(c_x, s_x, c_1x, s_1x):
    c_x = np.asarray(c_x, dtype=np.float32)
    s_x = np.asarray(s_x, dtype=np.float32)
    c_1x = np.asarray(c_1x, dtype=np.float32)
    s_1x = np.asarray(s_1x, dtype=np.float32)
    in_maps = []
    for core in range(8):
        s, h = divmod(core, 2)
        c1 = c_1x[s].reshape(C, HW)
        cxm = c_x[s].reshape(C, HW)
        if h == 1:
            c1 = np.concatenate([c1[:, QH:], c1[:, :QH]], axis=1)
            cxm = np.concatenate([cxm[:, QH:], cxm[:, :QH]], axis=1)
        in_maps.append(
            {
                "c1x": np.ascontiguousarray(c1.astype(np.float16)),
                "cx": np.ascontiguousarray(cxm.astype(np.float16)),
                "s1x": np.ascontiguousarray(s_1x[s].reshape(C, HW).astype(np.float16)),
                "sxt": np.ascontiguousarray(s_x[s].reshape(C, HW).T.astype(np.float16)),
            }
        )
    return in_maps


def _assemble(results):
    out = np.empty((4, C, 64, 64), np.float32)
    ov = out.reshape(4, C, HW)
    for core in range(8):
        s, h = divmod(core, 2)
        ov[s][:, h * QH : (h + 1) * QH] = results[core]["out"].T
    return out


def _run(in_maps, **kwargs):
    from concourse.bass_utils import run_bass_kernel_spmd

    return run_bass_kernel_spmd(_get_nc(), in_maps, core_ids=list(range(8)), **kwargs)


def kernel(c_x, s_x, c_1x, s_1x):
    res = _run(_prepare_in_maps(c_x, s_x, c_1x, s_1x))
    return _assemble(res.results)
